# revision 2
# baseline (speedup 1.0000x reference)
"""AffineCoupling (dense MLP) Trainium2 kernel.

Reference computation (B=16384, D=1024, HALF=512, HID=4096):
    a = z[:, 0::2]; b = z[:, 1::2]
    s = relu(a @ W1s + b1s) @ W2s + b2s
    t = relu(a @ W1t + b1t) @ W2t + b2t
    b_out = b * exp(s) + t
    logdet = s.sum(axis=1)
    z_out = interleave(a, b_out)

Strategy: data-parallel batch shard across 8 cores (2048 rows each), all
params replicated. On device everything is kept feature-major ("transposed",
[feature, batch]) so MLP biases are per-partition scalars and no on-device
transposes are needed; the host passes a^T / b^T slices and re-interleaves
the outputs. Matmuls run in bf16 (fp32 PSUM accumulation) at the full PE
rate; L1 and L2 of each MLP are software-pipelined per 128-row HID chunk so
the tensor engine never idles. logdet (a cross-partition sum) is a
ones-vector matmul.
"""

import numpy as np
import ml_dtypes

import concourse.bass as bass
import concourse.tile as tile
from concourse import bacc, mybir
from concourse import bass_utils
from concourse.bass import ts

AF = mybir.ActivationFunctionType
ALU = mybir.AluOpType

B, D, HID = 16384, 1024, 4096
HALF = D // 2            # 512
NCORES = 8
BC = B // NCORES         # 2048 batch rows per core
P = 128
TB = 512                 # batch tile (matmul moving free dim)
NB = BC // TB            # 4 batch tiles per core
KF = HALF // P           # 4 feature chunks (L1 contraction)
MH = HID // P            # 32 hidden chunks
MO = HALF // P           # 4 output chunks

F32 = mybir.dt.float32
BF16 = mybir.dt.bfloat16
NPBF = ml_dtypes.bfloat16

_CACHE: dict = {}


def _build_nc():
    from contextlib import ExitStack

    nc = bacc.Bacc("TRN2", target_bir_lowering=False, debug=False,
                   enable_asserts=False)

    at_d = nc.dram_tensor("at", [HALF, BC], BF16, kind="ExternalInput").ap()
    bt_d = nc.dram_tensor("bt", [HALF, BC], F32, kind="ExternalInput").ap()
    w1_d = {br: nc.dram_tensor(f"w1{br}", [HALF, HID], BF16,
                               kind="ExternalInput").ap() for br in "st"}
    w2_d = {br: nc.dram_tensor(f"w2{br}", [HID, HALF], BF16,
                               kind="ExternalInput").ap() for br in "st"}
    b1_d = {br: nc.dram_tensor(f"b1{br}", [P, MH], F32,
                               kind="ExternalInput").ap() for br in "st"}
    b2_d = {br: nc.dram_tensor(f"b2{br}", [P, MO], F32,
                               kind="ExternalInput").ap() for br in "st"}
    bout_d = nc.dram_tensor("boutt", [HALF, BC], F32, kind="ExternalOutput").ap()
    ld_d = nc.dram_tensor("logdet", [1, BC], F32, kind="ExternalOutput").ap()

    with tile.TileContext(nc) as tc, ExitStack() as ctx:
        wpool = ctx.enter_context(tc.tile_pool(name="w", bufs=1))
        cpool = ctx.enter_context(tc.tile_pool(name="c", bufs=1))
        apool = ctx.enter_context(tc.tile_pool(name="a", bufs=6))
        bpool = ctx.enter_context(tc.tile_pool(name="b", bufs=6))
        hpool = ctx.enter_context(tc.tile_pool(name="h", bufs=4))
        spool = ctx.enter_context(tc.tile_pool(name="s", bufs=6))
        epool = ctx.enter_context(tc.tile_pool(name="e", bufs=6))
        opool = ctx.enter_context(tc.tile_pool(name="o", bufs=4))
        psh = ctx.enter_context(tc.tile_pool(name="psh", bufs=3, space="PSUM"))
        pss = ctx.enter_context(tc.tile_pool(name="pss", bufs=1, space="PSUM"))
        psl = ctx.enter_context(tc.tile_pool(name="psl", bufs=1, space="PSUM"))

        # Persistent weights / biases, loaded once.
        w1 = {}
        w2 = {}
        for br in "st":
            for k in range(KF):
                t = wpool.tile([P, HID], BF16, tag=f"w1{br}{k}", name=f"w1{br}{k}")
                nc.sync.dma_start(t[:], w1_d[br][k * P:(k + 1) * P, :])
                w1[br, k] = t
            for k2 in range(MH):
                t = wpool.tile([P, HALF], BF16, tag=f"w2{br}{k2}", name=f"w2{br}{k2}")
                nc.sync.dma_start(t[:], w2_d[br][k2 * P:(k2 + 1) * P, :])
                w2[br, k2] = t
        b1 = {}
        b2 = {}
        for br in "st":
            t = cpool.tile([P, MH], F32, tag=f"b1{br}", name=f"b1{br}")
            nc.sync.dma_start(t[:], b1_d[br][:, :])
            b1[br] = t
            t = cpool.tile([P, MO], F32, tag=f"b2{br}", name=f"b2{br}")
            nc.sync.dma_start(t[:], b2_d[br][:, :])
            b2[br] = t
        ones = cpool.tile([P, 1], BF16, tag="ones", name="ones")
        nc.vector.memset(ones[:], 1.0)

        for n in range(NB):
            ncol = slice(n * TB, (n + 1) * TB)
            at_t = []
            bt_t = []
            for k in range(KF):
                t = apool.tile([P, TB], BF16, tag="at", name="at")
                nc.sync.dma_start(t[:], at_d[k * P:(k + 1) * P, ncol])
                at_t.append(t)
                t = bpool.tile([P, TB], F32, tag="bt", name="bt")
                nc.sync.dma_start(t[:], bt_d[k * P:(k + 1) * P, ncol])
                bt_t.append(t)

            s_tiles = []
            exp_tiles = []
            for br in "st":
                psS = [pss.tile([P, TB], F32, tag=f"pss{mo}", name=f"pss{mo}") for mo in range(MO)]
                # L1 (h = relu(a@W1+b1), chunk m) feeding L2 (accumulate
                # h-chunk into S), software-pipelined: L2 for chunk m-1 is
                # issued after L1 for chunk m so the PE never waits on ACT.
                pend = None
                for m in range(MH):
                    ph = psh.tile([P, TB], F32, tag="psh", name="psh")
                    for k in range(KF):
                        nc.tensor.matmul(ph[:], w1[br, k][:, ts(m, P)],
                                         at_t[k][:],
                                         start=(k == 0), stop=(k == KF - 1))
                    ht = hpool.tile([P, TB], BF16, tag="ht", name="ht")
                    nc.scalar.activation(ht[:], ph[:], AF.Relu,
                                         bias=b1[br][:, m:m + 1], scale=1.0)
                    if pend is not None:
                        pm, pht = pend
                        for mo in range(MO):
                            nc.tensor.matmul(psS[mo][:],
                                             w2[br, pm][:, ts(mo, P)], pht[:],
                                             start=(pm == 0), stop=False)
                    pend = (m, ht)
                pm, pht = pend
                for mo in range(MO):
                    nc.tensor.matmul(psS[mo][:], w2[br, pm][:, ts(mo, P)],
                                     pht[:], start=False, stop=True)

                if br == "s":
                    for mo in range(MO):
                        sb = spool.tile([P, TB], BF16, tag="sbf", name="sbf")
                        nc.scalar.activation(sb[:], psS[mo][:], AF.Identity,
                                             bias=b2["s"][:, mo:mo + 1],
                                             scale=1.0)
                        ex = epool.tile([P, TB], F32, tag="exps", name="exps")
                        nc.scalar.activation(ex[:], psS[mo][:], AF.Exp,
                                             bias=b2["s"][:, mo:mo + 1],
                                             scale=1.0)
                        s_tiles.append(sb)
                        exp_tiles.append(ex)
                else:
                    for mo in range(MO):
                        tmp = opool.tile([P, TB], F32, tag="tmp", name="tmp")
                        nc.vector.tensor_mul(tmp[:], bt_t[mo][:],
                                             exp_tiles[mo][:])
                        bo = opool.tile([P, TB], F32, tag="bout", name="bout")
                        nc.vector.scalar_tensor_tensor(
                            bo[:], psS[mo][:], b2["t"][:, mo:mo + 1], tmp[:],
                            op0=ALU.add, op1=ALU.add)
                        nc.sync.dma_start(bout_d[mo * P:(mo + 1) * P, ncol],
                                          bo[:])

            # logdet = column sums of s^T via ones-vector matmul; issued
            # after the t branch so ACT has long finished the s tiles.
            pld = psl.tile([1, TB], F32, tag="pld", name="pld")
            for mo in range(MO):
                nc.tensor.matmul(pld[:], ones[:], s_tiles[mo][:],
                                 start=(mo == 0), stop=(mo == MO - 1))
            ldt = opool.tile([1, TB], F32, tag="ld", name="ld")
            nc.vector.tensor_copy(ldt[:], pld[:])
            nc.sync.dma_start(ld_d[0:1, ncol], ldt[:])

    nc.compile()
    return nc


def _get_nc():
    if "nc" not in _CACHE:
        _CACHE["nc"] = _build_nc()
    return _CACHE["nc"]


def kernel(z, W1s, b1s, W2s, b2s, W1t, b1t, W2t, b2t):
    z = np.asarray(z, dtype=np.float32)
    a = z[:, 0::2]
    bmat = z[:, 1::2]
    AT = np.ascontiguousarray(a.T).astype(NPBF)       # [HALF, B]
    BT = np.ascontiguousarray(bmat.T)                 # [HALF, B] f32

    def prep_w(w):
        return np.ascontiguousarray(np.asarray(w, dtype=np.float32)).astype(NPBF)

    def prep_b1(b):
        return np.ascontiguousarray(np.asarray(b, np.float32).reshape(MH, P).T)

    def prep_b2(b):
        return np.ascontiguousarray(np.asarray(b, np.float32).reshape(MO, P).T)

    shared = {
        "w1s": prep_w(W1s), "w2s": prep_w(W2s),
        "w1t": prep_w(W1t), "w2t": prep_w(W2t),
        "b1s": prep_b1(b1s), "b2s": prep_b2(b2s),
        "b1t": prep_b1(b1t), "b2t": prep_b2(b2t),
    }
    in_maps = []
    for c in range(NCORES):
        sl = slice(c * BC, (c + 1) * BC)
        in_maps.append({
            "at": np.ascontiguousarray(AT[:, sl]),
            "bt": np.ascontiguousarray(BT[:, sl]),
            **shared,
        })

    nc = _get_nc()
    res = bass_utils.run_bass_kernel_spmd(nc, in_maps,
                                          core_ids=list(range(NCORES)))

    boutt = np.concatenate([r["boutt"] for r in res.results], axis=1)  # [HALF, B]
    logdet = np.concatenate([r["logdet"][0] for r in res.results])     # [B]

    z_out = np.empty_like(z)
    z_out[:, 0::2] = a
    z_out[:, 1::2] = boutt.T
    return z_out, logdet


# revision 3
# speedup vs baseline: 1.0433x; 1.0433x over previous
"""AffineCoupling (dense MLP) Trainium2 kernel.

Reference computation (B=16384, D=1024, HALF=512, HID=4096):
    a = z[:, 0::2]; b = z[:, 1::2]
    s = relu(a @ W1s + b1s) @ W2s + b2s
    t = relu(a @ W1t + b1t) @ W2t + b2t
    b_out = b * exp(s) + t
    logdet = s.sum(axis=1)
    z_out = interleave(a, b_out)

Strategy: data-parallel batch shard across 8 cores (2048 rows each), all
params replicated. On device everything is kept feature-major ("transposed",
[feature, batch]) so MLP biases are per-partition scalars and no on-device
transposes are needed; the host passes a^T / b^T slices and re-interleaves
the outputs. Matmuls run in bf16 (fp32 PSUM accumulation) at the full PE
rate; L1 and L2 of each MLP are software-pipelined per 128-row HID chunk so
the tensor engine never idles. logdet (a cross-partition sum) is a
ones-vector matmul.
"""

import numpy as np
import ml_dtypes

import concourse.bass as bass
import concourse.tile as tile
from concourse import bacc, mybir
from concourse import bass_utils
from concourse.bass import ts

AF = mybir.ActivationFunctionType
ALU = mybir.AluOpType

B, D, HID = 16384, 1024, 4096
HALF = D // 2            # 512
NCORES = 8
BC = B // NCORES         # 2048 batch rows per core
P = 128
TB = 512                 # batch tile (matmul moving free dim)
NB = BC // TB            # 4 batch tiles per core
KF = HALF // P           # 4 feature chunks (L1 contraction)
MH = HID // P            # 32 hidden chunks
MO = HALF // P           # 4 output chunks

F32 = mybir.dt.float32
BF16 = mybir.dt.bfloat16
NPBF = ml_dtypes.bfloat16

_CACHE: dict = {}


def _build_nc():
    from contextlib import ExitStack

    nc = bacc.Bacc("TRN2", target_bir_lowering=False, debug=False,
                   enable_asserts=False)

    at_d = nc.dram_tensor("at", [HALF, BC], BF16, kind="ExternalInput").ap()
    bt_d = nc.dram_tensor("bt", [HALF, BC], F32, kind="ExternalInput").ap()
    w1_d = {br: nc.dram_tensor(f"w1{br}", [HALF, HID], BF16,
                               kind="ExternalInput").ap() for br in "st"}
    w2_d = {br: nc.dram_tensor(f"w2{br}", [HID, HALF], BF16,
                               kind="ExternalInput").ap() for br in "st"}
    b1_d = {br: nc.dram_tensor(f"b1{br}", [P, MH], F32,
                               kind="ExternalInput").ap() for br in "st"}
    b2_d = {br: nc.dram_tensor(f"b2{br}", [P, MO], F32,
                               kind="ExternalInput").ap() for br in "st"}
    bout_d = nc.dram_tensor("boutt", [HALF, BC], F32, kind="ExternalOutput").ap()
    ld_d = nc.dram_tensor("logdet", [1, BC], F32, kind="ExternalOutput").ap()

    with tile.TileContext(nc) as tc, ExitStack() as ctx:
        wpool = ctx.enter_context(tc.tile_pool(name="w", bufs=1))
        cpool = ctx.enter_context(tc.tile_pool(name="c", bufs=1))
        apool = ctx.enter_context(tc.tile_pool(name="a", bufs=9))
        bpool = ctx.enter_context(tc.tile_pool(name="b", bufs=9))
        hpool = ctx.enter_context(tc.tile_pool(name="h", bufs=4))
        spool = ctx.enter_context(tc.tile_pool(name="s", bufs=6))
        epool = ctx.enter_context(tc.tile_pool(name="e", bufs=6))
        opool = ctx.enter_context(tc.tile_pool(name="o", bufs=4))
        psh = ctx.enter_context(tc.tile_pool(name="psh", bufs=3, space="PSUM"))
        pss = ctx.enter_context(tc.tile_pool(name="pss", bufs=1, space="PSUM"))
        psl = ctx.enter_context(tc.tile_pool(name="psl", bufs=1, space="PSUM"))

        # DMA issue order is tuned so the PE can start ASAP: the first
        # batch tile's a^T and the s-branch weights go first on the sync
        # HWDGE ring; the t-branch weights stream in parallel on the
        # scalar HWDGE ring (second physical ring).
        def load_ab(n):
            ncol = slice(n * TB, (n + 1) * TB)
            at_t = []
            bt_t = []
            for k in range(KF):
                t = apool.tile([P, TB], BF16, tag="at", name="at")
                nc.sync.dma_start(t[:], at_d[k * P:(k + 1) * P, ncol])
                at_t.append(t)
            for k in range(KF):
                t = bpool.tile([P, TB], F32, tag="bt", name="bt")
                nc.sync.dma_start(t[:], bt_d[k * P:(k + 1) * P, ncol])
                bt_t.append(t)
            return at_t, bt_t

        w1 = {}
        w2 = {}
        b1 = {}
        b2 = {}

        ab_next = load_ab(0)

        # s-branch weights on the sync ring, in consumption order; w1s in
        # column halves so L1 can start before the whole tile lands.
        HH = HID // 2
        for k in range(KF):
            t = wpool.tile([P, HID], BF16, tag=f"w1s{k}", name=f"w1s{k}")
            nc.sync.dma_start(t[:, 0:HH], w1_d["s"][k * P:(k + 1) * P, 0:HH])
            w1["s", k] = t
        t = cpool.tile([P, MH], F32, tag="b1s", name="b1s")
        nc.sync.dma_start(t[:], b1_d["s"][:, :])
        b1["s"] = t
        ones = cpool.tile([P, 1], BF16, tag="ones", name="ones")
        nc.vector.memset(ones[:], 1.0)
        for k in range(KF):
            nc.sync.dma_start(w1["s", k][:, HH:HID],
                              w1_d["s"][k * P:(k + 1) * P, HH:HID])
        for k2 in range(MH):
            t = wpool.tile([P, HALF], BF16, tag=f"w2s{k2}", name=f"w2s{k2}")
            nc.sync.dma_start(t[:], w2_d["s"][k2 * P:(k2 + 1) * P, :])
            w2["s", k2] = t
        t = cpool.tile([P, MO], F32, tag="b2s", name="b2s")
        nc.sync.dma_start(t[:], b2_d["s"][:, :])
        b2["s"] = t

        # t-branch weights on the scalar ring (parallel with the above).
        for k in range(KF):
            t = wpool.tile([P, HID], BF16, tag=f"w1t{k}", name=f"w1t{k}")
            nc.scalar.dma_start(t[:], w1_d["t"][k * P:(k + 1) * P, :])
            w1["t", k] = t
        t = cpool.tile([P, MH], F32, tag="b1t", name="b1t")
        nc.scalar.dma_start(t[:], b1_d["t"][:, :])
        b1["t"] = t
        for k2 in range(MH):
            t = wpool.tile([P, HALF], BF16, tag=f"w2t{k2}", name=f"w2t{k2}")
            nc.scalar.dma_start(t[:], w2_d["t"][k2 * P:(k2 + 1) * P, :])
            w2["t", k2] = t
        t = cpool.tile([P, MO], F32, tag="b2t", name="b2t")
        nc.scalar.dma_start(t[:], b2_d["t"][:, :])
        b2["t"] = t

        for n in range(NB):
            ncol = slice(n * TB, (n + 1) * TB)
            at_t, bt_t = ab_next
            if n + 1 < NB:
                ab_next = load_ab(n + 1)

            s_tiles = []
            exp_tiles = []
            for br in "st":
                psS = [pss.tile([P, TB], F32, tag=f"pss{mo}", name=f"pss{mo}") for mo in range(MO)]
                # L1 (h = relu(a@W1+b1), chunk m) feeding L2 (accumulate
                # h-chunk into S), software-pipelined: L2 for chunk m-1 is
                # issued after L1 for chunk m so the PE never waits on ACT.
                pend = None
                for m in range(MH):
                    ph = psh.tile([P, TB], F32, tag="psh", name="psh")
                    for k in range(KF):
                        nc.tensor.matmul(ph[:], w1[br, k][:, ts(m, P)],
                                         at_t[k][:],
                                         start=(k == 0), stop=(k == KF - 1))
                    ht = hpool.tile([P, TB], BF16, tag="ht", name="ht")
                    nc.scalar.activation(ht[:], ph[:], AF.Relu,
                                         bias=b1[br][:, m:m + 1], scale=1.0)
                    if pend is not None:
                        pm, pht = pend
                        for mo in range(MO):
                            nc.tensor.matmul(psS[mo][:],
                                             w2[br, pm][:, ts(mo, P)], pht[:],
                                             start=(pm == 0), stop=False)
                    pend = (m, ht)
                pm, pht = pend
                for mo in range(MO):
                    nc.tensor.matmul(psS[mo][:], w2[br, pm][:, ts(mo, P)],
                                     pht[:], start=False, stop=True)

                if br == "s":
                    for mo in range(MO):
                        sb = spool.tile([P, TB], BF16, tag="sbf", name="sbf")
                        nc.vector.tensor_scalar_add(sb[:], psS[mo][:],
                                                    b2["s"][:, mo:mo + 1])
                        ex = epool.tile([P, TB], F32, tag="exps", name="exps")
                        nc.scalar.activation(ex[:], psS[mo][:], AF.Exp,
                                             bias=b2["s"][:, mo:mo + 1],
                                             scale=1.0)
                        s_tiles.append(sb)
                        exp_tiles.append(ex)
                else:
                    for mo in range(MO):
                        tmp = opool.tile([P, TB], F32, tag="tmp", name="tmp")
                        nc.vector.tensor_mul(tmp[:], bt_t[mo][:],
                                             exp_tiles[mo][:])
                        bo = opool.tile([P, TB], F32, tag="bout", name="bout")
                        nc.vector.scalar_tensor_tensor(
                            bo[:], psS[mo][:], b2["t"][:, mo:mo + 1], tmp[:],
                            op0=ALU.add, op1=ALU.add)
                        nc.sync.dma_start(bout_d[mo * P:(mo + 1) * P, ncol],
                                          bo[:])

            # logdet = column sums of s^T via ones-vector matmul; issued
            # after the t branch so ACT has long finished the s tiles.
            pld = psl.tile([1, TB], F32, tag="pld", name="pld")
            for mo in range(MO):
                nc.tensor.matmul(pld[:], ones[:], s_tiles[mo][:],
                                 start=(mo == 0), stop=(mo == MO - 1))
            ldt = opool.tile([1, TB], F32, tag="ld", name="ld")
            nc.vector.tensor_copy(ldt[:], pld[:])
            nc.sync.dma_start(ld_d[0:1, ncol], ldt[:])

    nc.compile()
    return nc


def _get_nc():
    if "nc" not in _CACHE:
        _CACHE["nc"] = _build_nc()
    return _CACHE["nc"]


def kernel(z, W1s, b1s, W2s, b2s, W1t, b1t, W2t, b2t):
    z = np.asarray(z, dtype=np.float32)
    a = z[:, 0::2]
    bmat = z[:, 1::2]
    AT = np.ascontiguousarray(a.T).astype(NPBF)       # [HALF, B]
    BT = np.ascontiguousarray(bmat.T)                 # [HALF, B] f32

    def prep_w(w):
        return np.ascontiguousarray(np.asarray(w, dtype=np.float32)).astype(NPBF)

    def prep_b1(b):
        return np.ascontiguousarray(np.asarray(b, np.float32).reshape(MH, P).T)

    def prep_b2(b):
        return np.ascontiguousarray(np.asarray(b, np.float32).reshape(MO, P).T)

    shared = {
        "w1s": prep_w(W1s), "w2s": prep_w(W2s),
        "w1t": prep_w(W1t), "w2t": prep_w(W2t),
        "b1s": prep_b1(b1s), "b2s": prep_b2(b2s),
        "b1t": prep_b1(b1t), "b2t": prep_b2(b2t),
    }
    in_maps = []
    for c in range(NCORES):
        sl = slice(c * BC, (c + 1) * BC)
        in_maps.append({
            "at": np.ascontiguousarray(AT[:, sl]),
            "bt": np.ascontiguousarray(BT[:, sl]),
            **shared,
        })

    nc = _get_nc()
    res = bass_utils.run_bass_kernel_spmd(nc, in_maps,
                                          core_ids=list(range(NCORES)))

    boutt = np.concatenate([r["boutt"] for r in res.results], axis=1)  # [HALF, B]
    logdet = np.concatenate([r["logdet"][0] for r in res.results])     # [B]

    z_out = np.empty_like(z)
    z_out[:, 0::2] = a
    z_out[:, 1::2] = boutt.T
    return z_out, logdet
